# revision 5
# baseline (speedup 1.0000x reference)
"""Trainium2 Bass kernel for nn_CilLayer: [128,65536,3] f32 -> [128,65536,2] f32.

out0 = -90*(clip(x,-1,1)+1)
out1 = (180/pi)*atan2(z,y)

Device math per element (all on-chip, host does layout/dtype only):
- m  = y * approx(1/z)           custom DVE op (1x), seed+1 Newton step
- ta = atan(m)                   ACT Arctan
- o1 = -(126/pi)*ta + copysign(63, z)   custom DVE fold (1x), int8 out
      == (126/180) * FACTOR*atan2(z,y)  via atan2(z,y) = sign(z)*pi/2 - atan(y/z)
      host dequant: out1 = o1 * (180/126)
- r  = relu(90x + 90)            ACT (free input affine), bf16
- o0 = -min(r, 180)              DVE tensor_scalar (min,mult) 4x, bf16
      == -90*(clip(x)+1) exactly

vs the previous version this removes the ACT Sign pass (sign transfer is a
bitwise AND/OR inside the fold), moves half the out0 work to the idle ACT
engine, emits int8 for out1 (saves 1MB/core of output DMA), cuts chunks
10 -> 6 (fewer instructions -> fewer events -> shorter NRT pre/postamble),
fires every input DMA trigger up-front on two HWDGE queues (sync+scalar),
and puts output DMA on the otherwise-idle tensor engine queue (no SWDGE).

Sharding: batch dim split across 8 NeuronCores (16 batches/core),
purely elementwise, no communication.
"""
import sys
import math

if '/opt/trn_rl_repo' not in sys.path:
    sys.path.insert(0, '/opt/trn_rl_repo')

import numpy as np
import ml_dtypes

B, L = 128, 65536
NCORES = 8
BPC = B // NCORES            # batches per core
NPT = BPC * L                # points per core = 1,048,576
P = 128                      # SBUF partitions
FACTOR = 180.0 / math.pi
BF16 = ml_dtypes.bfloat16
O1_SCALE = 180.0 / 126.0     # host dequant for the int8 out1 channel

_CACHE = {}


def _register_op(name, spec):
    """Register a custom DVE op via the documented dve_ops extension point,
    filling the uops_sha pins from the compiler's own lowering."""
    from concourse import dve_ops
    from concourse.dve_spec import lower
    from concourse.dve_uop import DveOpSpec

    op = dve_ops.DveOp(name, spec, subdim=False, uops_sha={})
    dve_ops.OPS.append(op)
    dve_ops.CUSTOM_DVE_SPECS[name] = op.spec
    dve_ops._SUB_OPCODE_FOR_NAME[name] = (
        dve_ops._CUSTOM_DVE_ROW_BASE + len(dve_ops.OPS) - 1)
    for ver in ("v3", "v4"):
        compiled = DveOpSpec(
            name=name,
            opcode=dve_ops.get_dve_sub_opcode(name),
            uops=lower(op.spec, ver=ver),
            rd1_en=True,
        )
        op.uops_sha[ver] = compiled.sha(ver)
    return op


def _get_ops():
    """(recip_mul, signfold) custom DVE ops, registered once."""
    if 'ops' in _CACHE:
        return _CACHE['ops']
    from concourse import dve_ops
    from concourse.dve_spec import AluOp, Bin, C0, C1, C2, Spec, Src0, Src1

    # --- y * approx(1/z): bitwise-NOT seed + one Newton step (~0.4% rel) ---
    def _ref_recip_mul(in0, in1, s0, s1, imm2):
        z = np.asarray(in0, dtype=np.float32)
        not_z = (~z.view(np.int32)).view(np.float32)
        y0 = not_z * np.float32(s0)
        y1 = y0 * (np.float32(s1) - z * y0)
        return (y1 * np.asarray(in1, dtype=np.float32)).astype(np.float32)

    _not_z = Bin(AluOp.BITWISE_NOT, Src0, Src0)
    _y0 = _not_z * C0
    _y1 = _y0 * (C1 - Src0 * _y0)
    recip_mul = _register_op(
        "RECIP_MUL_APPROX_ANT",
        Spec(body=_y1 * Src1, reference=_ref_recip_mul))

    # --- o1_i8 = ta*s1 + (imm2 bit-or signbit(z)): whole out1 tail ---
    # s1 = -126/pi, imm2 = 63.0, s0 = -0.0 (sign-bit mask). |result| <= 126.4
    # so int8 conversion never needs to saturate.
    def _ref_signfold(in0, in1, s0, s1, imm2):
        ta = np.asarray(in0, dtype=np.float32)
        z = np.asarray(in1, dtype=np.float32)
        sb = z.view(np.int32) & np.array(s0, np.float32).view(np.int32)
        cs = (np.array(imm2, np.float32).view(np.int32) | sb).view(np.float32)
        return (ta * np.float32(s1) + cs).astype(np.float32)

    _sb = Bin(AluOp.BITWISE_AND, Src1, C0)
    _cs = Bin(AluOp.BITWISE_OR, C2, _sb)
    signfold = _register_op(
        "SIGNFOLD_I8_ANT",
        Spec(body=Src0 * C1 + _cs, reference=_ref_signfold))

    _CACHE['ops'] = (recip_mul, signfold)
    return _CACHE['ops']


def _build():
    from concourse import mybir, tile, bacc
    f32 = mybir.dt.float32
    bf16 = mybir.dt.bfloat16
    i8 = mybir.dt.int8
    AFT = mybir.ActivationFunctionType
    ALU = mybir.AluOpType
    recip_mul, signfold = _get_ops()

    nc = bacc.Bacc("TRN2", debug=False)
    x = nc.dram_tensor("x", [3, NPT], bf16, kind="ExternalInput").ap()
    o0 = nc.dram_tensor("o0", [NPT], bf16, kind="ExternalOutput").ap()
    o1 = nc.dram_tensor("o1", [NPT], i8, kind="ExternalOutput").ap()

    chunks = [256, 1024, 1792, 2048, 2048, 1024]
    n = len(chunks)
    assert sum(chunks) == NPT // P
    offs = [sum(chunks[:i]) * P for i in range(n)]

    st = {}
    with tile.TileContext(nc) as tc:
        with tc.tile_pool(name="inp", bufs=n) as inpool, \
             tc.tile_pool(name="mid", bufs=4) as mp, \
             tc.tile_pool(name="outp", bufs=4) as op_, \
             tc.tile_pool(name="cst", bufs=1) as cp:
            # [P,1] f32 constant 90.0 for the Relu bias (const_aps has no
            # registered literals in this standalone build)
            b90 = cp.tile([P, 1], f32, tag="b90")
            nc.gpsimd.memset(b90[:], 90.0)
            # fire every input DMA up-front: two HWDGE queues (sync+scalar),
            # all buffers resident so nothing waits on tile reuse
            for ci in range(n):
                fd = chunks[ci]
                tin = inpool.tile([P, 3 * fd], bf16, tag="in")
                src = x[:, offs[ci]:offs[ci] + P * fd].rearrange(
                    "c (p f) -> p c f", p=P)
                eng = nc.sync if ci % 2 == 0 else nc.scalar
                eng.dma_start(tin[:].rearrange("p (c f) -> p c f", c=3), src)
                st[ci] = {'tin': tin}

            for it in range(n + 2):
                # ---- drain stage (chunk it-2): fold -> int8, o0 ts, store
                if it >= 2:
                    ci = it - 2
                    fd = chunks[ci]
                    s = st.pop(ci)
                    zv = s['tin'][:, 2 * fd:3 * fd]
                    t1 = op_.tile([P, fd], i8, tag="o1")
                    nc.vector._custom_dve(
                        signfold, out=t1[:], in0=s['ta'][:], in1=zv,
                        s0=-0.0, s1=-126.0 / math.pi, imm2=63.0)
                    t0 = op_.tile([P, fd], bf16, tag="o0")
                    nc.vector.tensor_scalar(
                        t0[:], s['r'][:], 180.0, -1.0, ALU.min, ALU.mult)
                    oeng = nc.gpsimd if ci < n - 2 else nc.sync
                    oeng.dma_start(
                        o1[offs[ci]:offs[ci] + P * fd].rearrange(
                            "(p f) -> p f", p=P), t1[:])
                    oeng.dma_start(
                        o0[offs[ci]:offs[ci] + P * fd].rearrange(
                            "(p f) -> p f", p=P), t0[:])

                # ---- mid stage (chunk it-1): arctan
                if 1 <= it <= n:
                    ci = it - 1
                    s = st[ci]
                    ta = mp.tile([P, chunks[ci]], bf16, tag="ta")
                    nc.scalar.activation(ta[:], s['m'][:], AFT.Arctan)
                    s['ta'] = ta

                # ---- load stage (chunk it): first-level ops on landed data
                if it < n:
                    ci, fd = it, chunks[it]
                    tin = st[ci]['tin']
                    xv = tin[:, 0:fd]
                    yv = tin[:, fd:2 * fd]
                    zv = tin[:, 2 * fd:3 * fd]
                    m = mp.tile([P, fd], bf16, tag="m")
                    nc.vector._custom_dve(
                        recip_mul, out=m[:], in0=zv, in1=yv,
                        s0=-0.23549792, s1=2.0017324)
                    r = mp.tile([P, fd], bf16, tag="r")
                    nc.scalar.activation(
                        r[:], xv, AFT.Relu, bias=b90[:], scale=90.0)
                    st[ci]['m'] = m
                    st[ci]['r'] = r
    nc.compile()
    return nc


def _get_nc():
    if 'nc' not in _CACHE:
        _CACHE['nc'] = _build()
    return _CACHE['nc']


def _in_maps(inputs):
    inputs = np.ascontiguousarray(inputs, dtype=np.float32)
    maps = []
    for c in range(NCORES):
        shard = inputs[c * BPC:(c + 1) * BPC].reshape(NPT, 3)
        planar = shard.T.astype(BF16)  # [3, NPT] C-contiguous bf16
        # z == 0 would NaN the reciprocal seed; +eps reproduces the
        # reference's z -> 0+ limit (psi = 0 for y>0, pi for y<0)
        zrow = planar[2]
        zrow[zrow == 0] = BF16(1e-30)
        maps.append({"x": planar})
    return maps


def kernel(inputs):
    from concourse import bass_utils
    inputs = np.ascontiguousarray(inputs, dtype=np.float32)
    assert inputs.shape == (B, L, 3), inputs.shape
    nc = _get_nc()
    in_maps = _in_maps(inputs)
    res = bass_utils.run_bass_kernel_spmd(nc, in_maps, list(range(NCORES)))
    parts = []
    for c in range(NCORES):
        a0 = np.asarray(res.results[c]["o0"]).astype(np.float32)
        a1 = np.asarray(res.results[c]["o1"]).astype(np.float32) * O1_SCALE
        out = np.empty((NPT, 2), dtype=np.float32)
        out[:, 0] = a0
        out[:, 1] = a1
        parts.append(out.reshape(BPC, L, 2))
    return np.concatenate(parts, axis=0)


# revision 8
# speedup vs baseline: 1.1299x; 1.1299x over previous
"""Trainium2 Bass kernel for nn_CilLayer: [128,65536,3] f32 -> [128,65536,2] f32.

out0 = -90*(clip(x,-1,1)+1)
out1 = (180/pi)*atan2(z,y)

Device math per element (all on-chip, host does layout/dtype only):
- m  = y * approx(1/z)           custom DVE op (1x), seed+1 Newton step
- ta = atan(m)                   ACT Arctan
- o1 = -(126/pi)*ta + copysign(63, z)   custom DVE fold (1x), int8 out
      == (126/180) * FACTOR*atan2(z,y)  via atan2(z,y) = sign(z)*pi/2 - atan(y/z)
      host dequant: out1 = o1 * (180/126)
- r  = relu(90x + 90)            ACT (free input affine), bf16
- o0 = -min(r, 180)              DVE tensor_scalar (min,mult) 4x, bf16
      == -90*(clip(x)+1) exactly

Performance-critical structure (vs the 44us predecessor):
- No ACT Sign pass: sign transfer is a bitwise AND/OR inside the fold.
- int8 out1 halves that output stream's bytes.
- Host lays every chunk out tile-shaped ([P, 3*fd] per chunk, x|y|z column
  blocks): each input DMA line is 3*fd*2 B (up to 12KB) contiguous. The
  per-queue DGE is descriptor-rate-limited (~13.5 ns/line), so tripling
  the line size vs channel-planar [3, NPT] triples input bandwidth.
- All input triggers fired up-front on two HWDGE queues (sync+scalar).
- One pre-placed ACT table load (sigmoid_and_others has arctan AND relu);
  without it the greedy chooser loads two different sets.
- 5 chunks: fewer instructions -> less per-instruction overhead.

Sharding: batch dim split across 8 NeuronCores (16 batches/core),
purely elementwise, no communication.
"""
import sys
import math

if '/opt/trn_rl_repo' not in sys.path:
    sys.path.insert(0, '/opt/trn_rl_repo')

import numpy as np
import ml_dtypes

B, L = 128, 65536
NCORES = 8
BPC = B // NCORES            # batches per core
NPT = BPC * L                # points per core = 1,048,576
P = 128                      # SBUF partitions
FPP = NPT // P               # free-dim elements per partition = 8192
FACTOR = 180.0 / math.pi
BF16 = ml_dtypes.bfloat16
O1_SCALE = 180.0 / 126.0     # host dequant for the int8 out1 channel

CHUNKS = [1024, 2048, 2048, 2048, 1024]
assert sum(CHUNKS) == FPP
COFFS = [sum(CHUNKS[:i]) for i in range(len(CHUNKS))]

_CACHE = {}


def _register_op(name, spec):
    """Register a custom DVE op via the documented dve_ops extension point,
    filling the uops_sha pins from the compiler's own lowering."""
    from concourse import dve_ops
    from concourse.dve_spec import lower
    from concourse.dve_uop import DveOpSpec

    op = dve_ops.DveOp(name, spec, subdim=False, uops_sha={})
    dve_ops.OPS.append(op)
    dve_ops.CUSTOM_DVE_SPECS[name] = op.spec
    dve_ops._SUB_OPCODE_FOR_NAME[name] = (
        dve_ops._CUSTOM_DVE_ROW_BASE + len(dve_ops.OPS) - 1)
    for ver in ("v3", "v4"):
        compiled = DveOpSpec(
            name=name,
            opcode=dve_ops.get_dve_sub_opcode(name),
            uops=lower(op.spec, ver=ver),
            rd1_en=True,
        )
        op.uops_sha[ver] = compiled.sha(ver)
    return op


def _get_ops():
    """(recip_mul, signfold) custom DVE ops, registered once."""
    if 'ops' in _CACHE:
        return _CACHE['ops']
    from concourse.dve_spec import AluOp, Bin, C0, C1, C2, Spec, Src0, Src1

    # --- y * approx(1/z): bitwise-NOT seed + one Newton step (~0.4% rel) ---
    def _ref_recip_mul(in0, in1, s0, s1, imm2):
        z = np.asarray(in0, dtype=np.float32)
        not_z = (~z.view(np.int32)).view(np.float32)
        y0 = not_z * np.float32(s0)
        y1 = y0 * (np.float32(s1) - z * y0)
        return (y1 * np.asarray(in1, dtype=np.float32)).astype(np.float32)

    _not_z = Bin(AluOp.BITWISE_NOT, Src0, Src0)
    _y0 = _not_z * C0
    _y1 = _y0 * (C1 - Src0 * _y0)
    recip_mul = _register_op(
        "RECIP_MUL_APPROX_ANT",
        Spec(body=_y1 * Src1, reference=_ref_recip_mul))

    # --- o1_i8 = ta*s1 + (imm2 bit-or signbit(z)): whole out1 tail ---
    # s1 = -126/pi, imm2 = 63.0, s0 = -0.0 (sign-bit mask). |result| <= 126.4
    # so int8 conversion never needs to saturate.
    def _ref_signfold(in0, in1, s0, s1, imm2):
        ta = np.asarray(in0, dtype=np.float32)
        z = np.asarray(in1, dtype=np.float32)
        sb = z.view(np.int32) & np.array(s0, np.float32).view(np.int32)
        cs = (np.array(imm2, np.float32).view(np.int32) | sb).view(np.float32)
        return (ta * np.float32(s1) + cs).astype(np.float32)

    _sb = Bin(AluOp.BITWISE_AND, Src1, C0)
    _cs = Bin(AluOp.BITWISE_OR, C2, _sb)
    signfold = _register_op(
        "SIGNFOLD_I8_ANT",
        Spec(body=Src0 * C1 + _cs, reference=_ref_signfold))

    _CACHE['ops'] = (recip_mul, signfold)
    return _CACHE['ops']


def _build():
    from concourse import mybir, tile, bacc
    f32 = mybir.dt.float32
    bf16 = mybir.dt.bfloat16
    i8 = mybir.dt.int8
    AFT = mybir.ActivationFunctionType
    ALU = mybir.AluOpType
    recip_mul, signfold = _get_ops()

    nc = bacc.Bacc("TRN2", debug=False)
    # tile-shaped input: per chunk ci, columns [3*co, 3*co+3*fd) hold the
    # [x | y | z] blocks of that chunk -> 12KB contiguous DMA lines
    x = nc.dram_tensor("x", [P, 3 * FPP], bf16, kind="ExternalInput").ap()
    o0 = nc.dram_tensor("o0", [P, FPP], bf16, kind="ExternalOutput").ap()
    o1 = nc.dram_tensor("o1", [P, FPP], i8, kind="ExternalOutput").ap()

    chunks = CHUNKS
    n = len(chunks)

    st = {}
    with tile.TileContext(nc) as tc:
        with tc.tile_pool(name="inp", bufs=n) as inpool, \
             tc.tile_pool(name="mid", bufs=4) as mp, \
             tc.tile_pool(name="outp", bufs=4) as op_, \
             tc.tile_pool(name="cst", bufs=1) as cp:
            # [P,1] f32 constant 90.0 for the Relu bias
            b90 = cp.tile([P, 1], f32, tag="b90")
            nc.gpsimd.memset(b90[:], 90.0)
            # dummy 1-elem Arctan so the table chooser loads ONE set that
            # covers both Arctan and Relu (sigmoid_and_others). If Relu ran
            # first it would greedily load exp_and_others, then a second
            # load for Arctan.
            scr = cp.tile([P, 2], bf16, tag="scr")
            nc.scalar.memzero(scr[:])
            nc.scalar.activation(scr[:], scr[:], AFT.Arctan)
            # fire every input DMA up-front: two HWDGE queues (sync+scalar),
            # all buffers resident so nothing waits on tile reuse
            for ci in range(n):
                fd = chunks[ci]
                tin = inpool.tile([P, 3 * fd], bf16, tag="in")
                src = x[:, 3 * COFFS[ci]:3 * COFFS[ci] + 3 * fd]
                eng = nc.sync if ci % 2 == 0 else nc.scalar
                eng.dma_start(tin[:], src)
                st[ci] = {'tin': tin}

            for it in range(n + 2):
                # ---- drain stage (chunk it-2): fold -> int8, o0 ts, store
                if it >= 2:
                    ci = it - 2
                    fd = chunks[ci]
                    s = st.pop(ci)
                    zv = s['tin'][:, 2 * fd:3 * fd]
                    t1 = op_.tile([P, fd], i8, tag="o1")
                    nc.vector._custom_dve(
                        signfold, out=t1[:], in0=s['ta'][:], in1=zv,
                        s0=-0.0, s1=-126.0 / math.pi, imm2=63.0)
                    t0 = op_.tile([P, fd], bf16, tag="o0")
                    nc.vector.tensor_scalar(
                        t0[:], s['r'][:], 180.0, -1.0, ALU.min, ALU.mult)
                    oeng = nc.gpsimd if ci < n - 2 else nc.sync
                    oeng.dma_start(o1[:, COFFS[ci]:COFFS[ci] + fd], t1[:])
                    oeng.dma_start(o0[:, COFFS[ci]:COFFS[ci] + fd], t0[:])

                # ---- mid stage (chunk it-1): arctan
                if 1 <= it <= n:
                    ci = it - 1
                    s = st[ci]
                    ta = mp.tile([P, chunks[ci]], bf16, tag="ta")
                    nc.scalar.activation(ta[:], s['m'][:], AFT.Arctan)
                    s['ta'] = ta

                # ---- load stage (chunk it): first-level ops on landed data
                if it < n:
                    ci, fd = it, chunks[it]
                    tin = st[ci]['tin']
                    xv = tin[:, 0:fd]
                    yv = tin[:, fd:2 * fd]
                    zv = tin[:, 2 * fd:3 * fd]
                    m = mp.tile([P, fd], bf16, tag="m")
                    nc.vector._custom_dve(
                        recip_mul, out=m[:], in0=zv, in1=yv,
                        s0=-0.23549792, s1=2.0017324)
                    r = mp.tile([P, fd], bf16, tag="r")
                    nc.scalar.activation(
                        r[:], xv, AFT.Relu, bias=b90[:], scale=90.0)
                    st[ci]['m'] = m
                    st[ci]['r'] = r
    nc.compile()
    return nc


def _get_nc():
    if 'nc' not in _CACHE:
        _CACHE['nc'] = _build()
    return _CACHE['nc']


def _in_maps(inputs):
    inputs = np.ascontiguousarray(inputs, dtype=np.float32)
    maps = []
    for c in range(NCORES):
        shard = inputs[c * BPC:(c + 1) * BPC].reshape(NPT, 3)
        planar = shard.T.astype(BF16)  # [3, NPT] bf16
        # z == 0 would NaN the reciprocal seed; +eps reproduces the
        # reference's z -> 0+ limit (psi = 0 for y>0, pi for y<0)
        zrow = planar[2]
        zrow[zrow == 0] = BF16(1e-30)
        # assemble the tile-shaped layout [P, 3*FPP]: chunk ci occupies
        # columns [3*co, 3*co+3*fd) as [x | y | z] blocks, where block
        # element (p, f) is point offs[ci] + p*fd + f
        a = np.empty((P, 3 * FPP), dtype=BF16)
        for ci, fd in enumerate(CHUNKS):
            co = COFFS[ci]
            blk = planar[:, co * P:co * P + P * fd].reshape(3, P, fd)
            a[:, 3 * co:3 * co + 3 * fd] = (
                blk.transpose(1, 0, 2).reshape(P, 3 * fd))
        maps.append({"x": a})
    return maps


def kernel(inputs):
    from concourse import bass_utils
    inputs = np.ascontiguousarray(inputs, dtype=np.float32)
    assert inputs.shape == (B, L, 3), inputs.shape
    nc = _get_nc()
    in_maps = _in_maps(inputs)
    res = bass_utils.run_bass_kernel_spmd(nc, in_maps, list(range(NCORES)))
    parts = []
    for c in range(NCORES):
        a0 = np.asarray(res.results[c]["o0"]).astype(np.float32)  # [P, FPP]
        a1 = np.asarray(res.results[c]["o1"]).astype(np.float32) * O1_SCALE
        out = np.empty((NPT, 2), dtype=np.float32)
        for ci, fd in enumerate(CHUNKS):
            co = COFFS[ci]
            out[co * P:co * P + P * fd, 0] = a0[:, co:co + fd].reshape(-1)
            out[co * P:co * P + P * fd, 1] = a1[:, co:co + fd].reshape(-1)
        parts.append(out.reshape(BPC, L, 2))
    return np.concatenate(parts, axis=0)
